# revision 1
# baseline (speedup 1.0000x reference)
"""CorrelationLayer1D Trainium2 kernel.

out[b,d,h,w] = sum_c x1[b,c,h,w] * x2[b,c,h,w-80+d]  (zero where index < 0)
B=8, C=128, H=160, W=320, D=81 (MAX_DISP=40, pad=80).

Sharding: data-parallel over batch, one batch element per NeuronCore (8 cores).

Per-core algorithm (all on device):
  For each h and each 64-wide w-chunk k, the TensorEngine computes the local
  Gram rectangle  q[m',n'] = sum_c x1[c,64k+m'] * x2pad[c,64k+n']  where
  x2pad is x2 left-padded with 80 zero columns.  The output band is the
  diagonals  out[d, 64k+m'] = q[m', m'+d], d in [0,81).
  Diagonal extraction cannot be done by compute engines (no per-partition
  offsets) nor by SBUF-side DMA APs (DGE resets the flat-step remainder at
  descriptor-run boundaries), but DRAM-side DMA APs with arbitrary outer
  strides and contiguous inner runs DO work.  So: bounce q to DRAM, re-load
  with a diagonal AP giving S[w-part, d-free], transpose on the PE via an
  identity matmul to T[d-part, w-free], and store with w contiguous.
"""

import numpy as np

B, C, H, W = 8, 128, 160, 320
D = 81
PAD = 80
MK = 64          # w-chunk width (matmul M)
NK = MK + PAD    # gram rectangle width (144)
NCHUNK = W // MK # 5
NH = 8           # h-group size
NGROUP = H // NH # 20
WP = W + PAD     # padded x2 row width (400)

_CACHE = {}


def _build_nc(repeat=1):
    import concourse.bass as bass
    import concourse.bacc as bacc
    import concourse.mybir as mybir
    from concourse import tile

    f32 = mybir.dt.float32
    nc = bacc.Bacc()

    x1 = nc.dram_tensor("x1", [C, H, W], f32, kind="ExternalInput")
    x2 = nc.dram_tensor("x2", [C, H, W], f32, kind="ExternalInput")
    ident = nc.dram_tensor("ident", [128, 128], f32, kind="ExternalInput")
    out = nc.dram_tensor("out", [D, H, W], f32, kind="ExternalOutput")

    with tile.TileContext(nc) as tc:
        with (
            tc.tile_pool(name="inpool", bufs=2) as inpool,
            tc.tile_pool(name="qpool", bufs=3) as qpool,
            tc.tile_pool(name="spool", bufs=3) as spool,
            tc.tile_pool(name="tpool", bufs=2) as tpool,
            tc.tile_pool(name="const", bufs=1) as constpool,
            tc.tile_pool(name="psq", bufs=4, space=bass.MemorySpace.PSUM) as psq,
            tc.tile_pool(name="pst", bufs=4, space=bass.MemorySpace.PSUM) as pst,
            tc.tile_pool(name="qdram", bufs=4, space="DRAM") as qdram,
        ):
            id_t = constpool.tile([128, 128], f32)
            nc.sync.dma_start(id_t[:, :], ident[:, :])

            for g in range(NGROUP * repeat):
                g = g % NGROUP
                h0 = g * NH
                # ---- load inputs for this h-group ----
                x1_t = inpool.tile([C, NH, W], f32, tag="x1t")
                nc.sync.dma_start(x1_t[:, :, :], x1[:, h0 : h0 + NH, :])
                # x2 goes into a padded layout: [C, NH, WP], first PAD cols zero
                x2_t = inpool.tile([C, NH, WP], f32, tag="x2t")
                nc.vector.memset(x2_t[:, :, 0:PAD], 0.0)
                nc.sync.dma_start(x2_t[:, :, PAD:WP], x2[:, h0 : h0 + NH, :])

                t_t = tpool.tile([D, NH, W], f32, tag="t")
                for k in range(NCHUNK):
                    # ---- gram rectangles for all h in group ----
                    q_t = qpool.tile([MK, NH, NK], f32, tag="q")
                    for hh in range(NH):
                        q_ps = psq.tile([MK, NK], f32, tag="qps")
                        nc.tensor.matmul(
                            q_ps[:, :],
                            x1_t[:, hh, k * MK : k * MK + MK],
                            x2_t[:, hh, k * MK : k * MK + NK],
                        )
                        nc.vector.tensor_copy(q_t[:, hh, :], q_ps[:, :])
                    # ---- bounce to DRAM ----
                    q_d = qdram.tile([MK, NH, NK], f32, tag="qd")
                    nc.sync.dma_start(q_d[:, :, :], q_t[:, :, :])
                    # ---- diagonal re-load: S[m', hh, e] = q_d[m', hh, m'+e] ----
                    s_t = spool.tile([MK, NH, D], f32, tag="s")
                    diag_src = bass.AP(
                        q_d.tensor,
                        q_d.offset,
                        [[NH * NK + 1, MK], [NK, NH], [1, D]],
                    )
                    nc.sync.dma_start(s_t[:, :, :], diag_src)
                    # ---- transpose S -> T via identity matmul, stash in sb ----
                    for hh in range(NH):
                        t_ps = pst.tile([D, MK], f32, tag="tps")
                        nc.tensor.matmul(
                            t_ps[:, :],
                            s_t[:, hh, :],
                            id_t[0:MK, 0:MK],
                        )
                        nc.vector.tensor_copy(
                            t_t[:, hh, k * MK : k * MK + MK], t_ps[:, :]
                        )
                # ---- store the whole h-group ----
                nc.sync.dma_start(out[:, h0 : h0 + NH, :], t_t[:, :, :])

    nc.compile()
    return nc


def _get_nc():
    if "nc" not in _CACHE:
        _CACHE["nc"] = _build_nc()
    return _CACHE["nc"]


def kernel(x_1: np.ndarray, x_2: np.ndarray) -> np.ndarray:
    from concourse.bass_utils import run_bass_kernel_spmd

    nc = _get_nc()
    x_1 = np.ascontiguousarray(x_1, dtype=np.float32)
    x_2 = np.ascontiguousarray(x_2, dtype=np.float32)
    ident = np.eye(128, dtype=np.float32)
    in_maps = [
        {"x1": x_1[b], "x2": x_2[b], "ident": ident} for b in range(B)
    ]
    res = run_bass_kernel_spmd(nc, in_maps, list(range(B)))
    return np.stack([res.results[b]["out"] for b in range(B)], axis=0)



# revision 2
# speedup vs baseline: 1.5667x; 1.5667x over previous
"""CorrelationLayer1D Trainium2 kernel.

out[b,d,h,w] = sum_c x1[b,c,h,w] * x2[b,c,h,w-80+d]  (zero where index < 0)
B=8, C=128, H=160, W=320, D=81 (MAX_DISP=40, pad=80).

Sharding: data-parallel over batch, one batch element per NeuronCore (8 cores).

Per-core algorithm:
  Inputs are fed as bf16 (host-cast; the 2e-2 rel-err budget dwarfs bf16
  noise and it halves input HBM traffic + runs the PE at 1 cycle/row
  instead of fp32's 4).  For each h and each 64-wide w-chunk k the
  TensorEngine computes the local Gram rectangle
      q[m, j] = sum_c x1[c, 64k+m] * x2pad[c, 64k+j],  j in [0,144)
  where x2pad is x2 left-padded with 80 zero columns.  The output band is
  the diagonals out[d, 64k+m] = q[m, m+d], d in [0,81).

  Diagonal extraction needs a per-partition (-m) shift, which no compute
  engine can do -- but a DRAM-side DMA access pattern CAN: writing q with
  partition stride (H*JP - 1) instead of H*JP lands q[m, hh, j] at
  buf[64k+m, h, j-m+63], i.e. the de-skew is folded into the output DMA.
  buf has a padded last axis JP=208 so the skewed writes never collide;
  buf[w, h, 63:144] is exactly out[.., h, w] transposed, which the host
  slices and transposes during the gather (no device work, no reload).

  Gram matmuls for hh-pairs are packed into 128-partition PSUM tiles
  (even hh -> partitions 0:64, odd -> 64:128) so the PSUM->SBUF copies
  run at full DVE width.
"""

import numpy as np

B, C, H, W = 8, 128, 160, 320
D = 81
PAD = 80
MK = 64          # w-chunk width (matmul M)
NK = MK + PAD    # gram rectangle width (144)
NCHUNK = W // MK # 5
NH = 8           # h-group size
NGROUP = H // NH # 20
WP = W + PAD     # padded x2 row width (400)
JP = 208         # padded skewed-j axis: j-m+63 in [0, 207]
JOFF = 63        # j' = j - m + JOFF

_CACHE = {}


def _build_nc():
    import concourse.bass as bass
    import concourse.bacc as bacc
    import concourse.mybir as mybir
    from concourse import tile

    f32 = mybir.dt.float32
    bf16 = mybir.dt.bfloat16
    nc = bacc.Bacc()

    x1 = nc.dram_tensor("x1", [C, H, W], bf16, kind="ExternalInput")
    x2 = nc.dram_tensor("x2", [C, H, W], bf16, kind="ExternalInput")
    out4 = nc.dram_tensor("out4", [W, H, JP], f32, kind="ExternalOutput")
    out4_t = out4[:, :, :].tensor

    with tile.TileContext(nc) as tc:
        with (
            tc.tile_pool(name="inpool", bufs=2) as inpool,
            tc.tile_pool(name="qpool", bufs=3) as qpool,
            tc.tile_pool(name="psq", bufs=2, space=bass.MemorySpace.PSUM) as psq,
        ):
            for g in range(NGROUP):
                h0 = g * NH
                # ---- load inputs for this h-group ----
                x1_t = inpool.tile([C, NH, W], bf16, tag="x1t")
                nc.gpsimd.dma_start(x1_t[:, :, :], x1[:, h0 : h0 + NH, :])
                # x2 goes into a padded layout: [C, NH, WP], first PAD cols zero
                x2_t = inpool.tile([C, NH, WP], bf16, tag="x2t")
                nc.vector.memset(x2_t[:, :, 0:PAD], 0.0)
                nc.gpsimd.dma_start(x2_t[:, :, PAD:WP], x2[:, h0 : h0 + NH, :])

                for k in range(NCHUNK):
                    w0 = k * MK
                    # q_t partition p = parity*64 + m, free (b, j), hh = 2b+parity
                    q_t = qpool.tile([128, 4, NK], f32, tag="q")
                    for t in range(2):
                        ps = psq.tile([128, 2, 512], f32, tag=f"ps{t}")
                        for bb in range(2):
                            b = 2 * t + bb
                            for parity in range(2):
                                hh = 2 * b + parity
                                nc.tensor.matmul(
                                    ps[parity * 64 : parity * 64 + 64, bb, 0:NK],
                                    x1_t[:, hh, w0 : w0 + MK],
                                    x2_t[:, hh, w0 : w0 + NK],
                                )
                        nc.vector.tensor_copy(
                            q_t[:, 2 * t : 2 * t + 2, :], ps[:, :, 0:NK]
                        )
                    # ---- skewed store: q[m,hh,j] -> out4[w0+m, h0+hh, j-m+63]
                    for parity in range(2):
                        dst = bass.AP(
                            out4_t,
                            w0 * H * JP + (h0 + parity) * JP + JOFF,
                            [[H * JP - 1, MK], [2 * JP, 4], [1, NK]],
                        )
                        nc.sync.dma_start(
                            dst, q_t[parity * 64 : parity * 64 + 64, :, :]
                        )

    nc.compile()
    return nc


def _get_nc():
    if "nc" not in _CACHE:
        _CACHE["nc"] = _build_nc()
    return _CACHE["nc"]


def kernel(x_1: np.ndarray, x_2: np.ndarray) -> np.ndarray:
    import ml_dtypes
    from concourse.bass_utils import run_bass_kernel_spmd

    nc = _get_nc()
    xb1 = np.ascontiguousarray(x_1).astype(ml_dtypes.bfloat16)
    xb2 = np.ascontiguousarray(x_2).astype(ml_dtypes.bfloat16)
    in_maps = [{"x1": xb1[b], "x2": xb2[b]} for b in range(B)]
    res = run_bass_kernel_spmd(nc, in_maps, list(range(B)))
    out = np.empty((B, D, H, W), dtype=np.float32)
    for b in range(B):
        buf = res.results[b]["out4"]  # [W, H, JP]
        out[b] = buf[:, :, JOFF : JOFF + D].transpose(2, 1, 0)
    return out


# revision 5
# speedup vs baseline: 2.3714x; 1.5137x over previous
"""CorrelationLayer1D Trainium2 kernel.

out[b,d,h,w] = sum_c x1[b,c,h,w] * x2[b,c,h,w-80+d]  (zero where index < 0)
B=8, C=128, H=160, W=320, D=81 (MAX_DISP=40, pad=80).

Sharding: data-parallel over batch, one batch element per NeuronCore (8 cores).

Per-core algorithm:
  Inputs are fed as bf16 (host-cast; the 2e-2 rel-err budget dwarfs bf16
  noise, it halves input HBM traffic, and bf16 streams the PE at full
  rate).  For each h-row and each w-chunk the TensorEngine computes the
  local Gram rectangle
      q[m, j] = sum_c x1[c, w0+m] * x2pad[c, w0+j]
  where x2pad is x2 left-padded with 80 zero columns.  The output band is
  the diagonals out[d, w0+m] = q[m, m+d], d in [0,81).

  Chunks are 128 wide (PE-stationary max) with a 64 tail: per-instruction
  cost dominates matmul time on this part, so fewer/bigger matmuls win.
  The gram of a 128-chunk is the fused gram of two 64-chunks with zero
  wasted moving columns (their x2 windows overlap).

  Diagonal extraction needs a per-partition (-m) shift, which no compute
  engine can do -- but a DRAM-side DMA access pattern CAN: writing q with
  partition stride (H*JP - 1) instead of H*JP lands q[m, hh, j] at
  buf[w0+m, h, j-m+63].  buf has a padded last axis JP=208 so the skewed
  writes never collide; buf[w, h, 63:144] is exactly out[.., h, w]
  transposed, which the host slices/casts/transposes during the gather.
  For 128-chunks the store is split per partition-half with trimmed
  j-windows ([0,144) for m<64, [64,208) for m>=64, both containing the
  band) so the bounce traffic stays at the 64-chunk level.  buf is bf16
  (outputs rounded to bf16: ~0.2% rel err, well inside the gate).
"""

import numpy as np

B, C, H, W = 8, 128, 160, 320
D = 81
PAD = 80
NH = 8           # h-group size
NGROUP = H // NH # 20
WP = W + PAD     # padded x2 row width (400)
JP = 208         # padded skewed-j axis: j-m+63 in [0, 207]
JOFF = 63        # j' = j - m_local + JOFF  (m_local = m % 64)

_CACHE = {}


def _build_nc():
    import concourse.bass as bass
    import concourse.bacc as bacc
    import concourse.mybir as mybir
    from concourse import tile

    f32 = mybir.dt.float32
    bf16 = mybir.dt.bfloat16
    nc = bacc.Bacc()

    x1 = nc.dram_tensor("x1", [C, H, W], bf16, kind="ExternalInput")
    x2 = nc.dram_tensor("x2", [C, H, W], bf16, kind="ExternalInput")
    out4 = nc.dram_tensor("out4", [W, H, JP], bf16, kind="ExternalOutput")
    out4_t = out4[:, :, :].tensor

    with tile.TileContext(nc) as tc:
        with (
            tc.tile_pool(name="inpool", bufs=2) as inpool,
            tc.tile_pool(name="qpool", bufs=3) as qpool,
            tc.tile_pool(name="psq", bufs=8, space=bass.MemorySpace.PSUM) as psq,
        ):
            for g in range(NGROUP):
                h0 = g * NH
                # ---- load inputs for this h-group (HWDGE via scalar) ----
                x1_t = inpool.tile([C, NH, W], bf16, tag="x1t")
                nc.scalar.dma_start(x1_t[:, :, :], x1[:, h0 : h0 + NH, :])
                # x2 goes into a padded layout: [C, NH, WP], first PAD cols zero
                x2_t = inpool.tile([C, NH, WP], bf16, tag="x2t")
                nc.vector.memset(x2_t[:, :, 0:PAD], 0.0)
                nc.scalar.dma_start(x2_t[:, :, PAD:WP], x2[:, h0 : h0 + NH, :])

                # chunks: [0,128) [128,256) [256,320)
                for k, (w0, mk) in enumerate([(0, 128), (128, 128), (256, 64)]):
                    nk = mk + PAD
                    q_t = qpool.tile([128, NH, nk], bf16, tag=f"q{k}")
                    for hh in range(NH):
                        ps = psq.tile([128, 512], f32, tag="ps")
                        nc.tensor.matmul(
                            ps[0:mk, 0:nk],
                            x1_t[:, hh, w0 : w0 + mk],
                            x2_t[:, hh, w0 : w0 + nk],
                        )
                        if hh % 2 == 0:
                            nc.vector.tensor_copy(q_t[0:mk, hh, :], ps[0:mk, 0:nk])
                        else:
                            nc.scalar.copy(q_t[0:mk, hh, :], ps[0:mk, 0:nk])
                    # ---- skewed store: q[m,hh,j] -> out4[w0+m, h0+hh, j-m%64+63]
                    if mk == 128:
                        # per-half trimmed j-windows, both contain the band
                        for half, jlo in ((0, 0), (1, 64)):
                            # j - m = (jlo + jr) - (half*64 + m') = jr - m'
                            # since jlo == half*64, so the offset is uniform.
                            dst = bass.AP(
                                out4_t,
                                (w0 + half * 64) * H * JP + h0 * JP + JOFF,
                                [[H * JP - 1, 64], [JP, NH], [1, 144]],
                            )
                            nc.sync.dma_start(
                                dst,
                                q_t[half * 64 : half * 64 + 64, :, jlo : jlo + 144],
                            )
                    else:
                        dst = bass.AP(
                            out4_t,
                            w0 * H * JP + h0 * JP + JOFF,
                            [[H * JP - 1, 64], [JP, NH], [1, 144]],
                        )
                        nc.sync.dma_start(dst, q_t[0:64, :, :])

    nc.compile()
    return nc


def _get_nc():
    if "nc" not in _CACHE:
        _CACHE["nc"] = _build_nc()
    return _CACHE["nc"]


def kernel(x_1: np.ndarray, x_2: np.ndarray) -> np.ndarray:
    import ml_dtypes
    from concourse.bass_utils import run_bass_kernel_spmd

    nc = _get_nc()
    xb1 = np.ascontiguousarray(x_1).astype(ml_dtypes.bfloat16)
    xb2 = np.ascontiguousarray(x_2).astype(ml_dtypes.bfloat16)
    in_maps = [{"x1": xb1[b], "x2": xb2[b]} for b in range(B)]
    res = run_bass_kernel_spmd(nc, in_maps, list(range(B)))
    out = np.empty((B, D, H, W), dtype=np.float32)
    for b in range(B):
        buf = res.results[b]["out4"]  # [W, H, JP] bf16
        out[b] = buf[:, :, JOFF : JOFF + D].transpose(2, 1, 0).astype(np.float32)
    return out
